# revision 1
# baseline (speedup 1.0000x reference)
"""Trainium2 Bass kernel for nn_Head (sparse attention head).

Computation (per batch b):
    K = X @ Wk; Q = X @ Wq; V = X @ Wv                       # [T, HS]
    S = Q K^T / sqrt(HS)                                     # [T, T]
    A = softmax_row(where(dag==0, -inf, S))                  # row-wise over keys
    out[j, h] = sum_i A[i, j] V[i, h]   (transposed AV)      # [T, HS]
    return swish(out)

Sharding over 8 NeuronCores: core = (b, h) with b = batch (4), h = query-row
half (2).  Each core computes its 2048-query slice: projections, masked
softmax numerator U = exp(S/8) * dag (mask applied multiplicatively after
exp on DVE, fused with the row-sum), folds the softmax denominator into the
V stationary operand, and produces the partial transposed-AV output
OT_partial[h, j] = sum_{i in shard} U[i,j] * (V[i,h]/l_i) * 1024.
Host sums the two partials per batch, divides by 1024, transposes, applies
swish.
"""

import sys

for _p in ("/opt/trn_rl_repo",):
    if _p not in sys.path:
        sys.path.append(_p)

import numpy as np

import concourse.bacc as bacc
import concourse.mybir as mybir
import concourse.tile as tile
from concourse.bass_utils import run_bass_kernel_spmd

B, T, D, HS = 4, 4096, 512, 64
TH = T // 2          # query rows per core
P = 128              # partitions
NB = TH // P         # 16 i-blocks per core
NCC = D // P         # 4 contraction chunks over D
NJ = 512             # matmul moving free dim
VSCALE = 1024.0      # fp16 dynamic-range scale folded into V/l

F16 = mybir.dt.float16
F32 = mybir.dt.float32
AF = mybir.ActivationFunctionType
ALU = mybir.AluOpType

_CACHE = {}


def _build():
    if "nc" in _CACHE:
        return _CACHE["nc"]

    nc = bacc.Bacc("TRN2", target_bir_lowering=False, debug=False)

    xt_d = nc.dram_tensor("xt", [D, T], F16, kind="ExternalInput").ap()
    xtq_d = nc.dram_tensor("xtq", [D, TH], F16, kind="ExternalInput").ap()
    m_d = nc.dram_tensor("m", [TH, T], F16, kind="ExternalInput").ap()
    wk_d = nc.dram_tensor("wk", [D, HS], F16, kind="ExternalInput").ap()
    wq_d = nc.dram_tensor("wq", [D, HS], F16, kind="ExternalInput").ap()
    wv_d = nc.dram_tensor("wv", [D, HS], F16, kind="ExternalInput").ap()
    ot_d = nc.dram_tensor("ot", [HS, T], F32, kind="ExternalOutput").ap()

    with tile.TileContext(nc) as tc:
        with tc.tile_pool(name="persist", bufs=1) as pp:
            kt = pp.tile([HS, T], F16, tag="kt")         # K^T
            qt = pp.tile([HS, TH], F16, tag="qt")        # Q^T (shard rows)
            v = pp.tile([P, NB * HS], F16, tag="v")      # V rows (shard)
            vt = pp.tile([P, NB * HS], F16, tag="vt")    # V/l * VSCALE

            # ---- phase A: load X^T / weights, compute K^T, Q^T, V ----
            with (
                tc.tile_pool(name="phA", bufs=1) as pA,
                tc.tile_pool(name="psA", bufs=2, space="PSUM") as psA,
            ):
                xt = pA.tile([P, NCC * T], F16, tag="xt")
                xtq = pA.tile([P, NCC * TH], F16, tag="xtq")
                wk = pA.tile([P, NCC * HS], F16, tag="wk")
                wq = pA.tile([P, NCC * HS], F16, tag="wq")
                wv = pA.tile([P, NCC * HS], F16, tag="wv")
                for ci in range(NCC):
                    cs = slice(ci * P, (ci + 1) * P)
                    nc.sync.dma_start(xt[:, ci * T:(ci + 1) * T], xt_d[cs, :])
                    nc.sync.dma_start(xtq[:, ci * TH:(ci + 1) * TH], xtq_d[cs, :])
                    nc.sync.dma_start(wk[:, ci * HS:(ci + 1) * HS], wk_d[cs, :])
                    nc.sync.dma_start(wq[:, ci * HS:(ci + 1) * HS], wq_d[cs, :])
                    nc.sync.dma_start(wv[:, ci * HS:(ci + 1) * HS], wv_d[cs, :])

                for j0 in range(0, T, NJ):
                    ktp = psA.tile([HS, NJ], F32, tag="ktp")
                    for ci in range(NCC):
                        nc.tensor.matmul(
                            ktp[:],
                            wk[:, ci * HS:(ci + 1) * HS],
                            xt[:, ci * T + j0: ci * T + j0 + NJ],
                            start=(ci == 0),
                            stop=(ci == NCC - 1),
                        )
                    nc.scalar.copy(kt[:, j0:j0 + NJ], ktp[:])

                for j0 in range(0, TH, NJ):
                    qtp = psA.tile([HS, NJ], F32, tag="ktp")
                    for ci in range(NCC):
                        nc.tensor.matmul(
                            qtp[:],
                            wq[:, ci * HS:(ci + 1) * HS],
                            xtq[:, ci * TH + j0: ci * TH + j0 + NJ],
                            start=(ci == 0),
                            stop=(ci == NCC - 1),
                        )
                    nc.scalar.copy(qt[:, j0:j0 + NJ], qtp[:])

                for k in range(NB):
                    vp = psA.tile([P, HS], F32, tag="vp")
                    for ci in range(NCC):
                        nc.tensor.matmul(
                            vp[:],
                            xtq[:, ci * TH + k * P: ci * TH + (k + 1) * P],
                            wv[:, ci * HS:(ci + 1) * HS],
                            start=(ci == 0),
                            stop=(ci == NCC - 1),
                        )
                    nc.scalar.copy(v[:, k * HS:(k + 1) * HS], vp[:])

            # ---- phase B: per i-block scores, exp, mask+rowsum ----
            ctx_big = tc.tile_pool(name="big", bufs=1)
            bigp = ctx_big.__enter__()
            u = bigp.tile([P, NB * T], F16, tag="u")     # masked exp(S/8)
            with (
                tc.tile_pool(name="phB", bufs=3) as pB,
                tc.tile_pool(name="phBl", bufs=2) as pBl,
                tc.tile_pool(name="psB", bufs=2, space="PSUM") as psB,
            ):
                for k in range(NB):
                    l_halves = []
                    for jh in range(2):
                        sp = psB.tile([P, TH], F32, tag="s")
                        for jq in range(4):
                            nc.tensor.matmul(
                                sp[:, jq * NJ:(jq + 1) * NJ],
                                qt[:, k * P:(k + 1) * P],
                                kt[:, jh * TH + jq * NJ: jh * TH + (jq + 1) * NJ],
                                start=True,
                                stop=True,
                            )
                        er = pB.tile([P, TH], F16, tag="eraw")
                        nc.scalar.activation(er[:], sp[:], AF.Exp, scale=0.125)
                        mk = pB.tile([P, TH], F16, tag="mask")
                        nc.sync.dma_start(
                            mk[:], m_d[k * P:(k + 1) * P, jh * TH:(jh + 1) * TH]
                        )
                        l_acc = pBl.tile([P, 1], F32, tag=f"l{jh}")
                        nc.vector.scalar_tensor_tensor(
                            out=u[:, k * T + jh * TH: k * T + (jh + 1) * TH],
                            in0=er[:],
                            scalar=1.0,
                            in1=mk[:],
                            op0=ALU.mult,
                            op1=ALU.mult,
                            accum_out=l_acc[:],
                        )
                        l_halves.append(l_acc)
                    l_tot = pBl.tile([P, 1], F32, tag="lt")
                    nc.vector.tensor_tensor(
                        out=l_tot[:], in0=l_halves[0][:], in1=l_halves[1][:],
                        op=ALU.add,
                    )
                    rl = pBl.tile([P, 1], F32, tag="rl")
                    nc.vector.reciprocal(rl[:], l_tot[:])
                    nc.vector.tensor_scalar(
                        out=vt[:, k * HS:(k + 1) * HS],
                        in0=v[:, k * HS:(k + 1) * HS],
                        scalar1=rl[:],
                        scalar2=VSCALE,
                        op0=ALU.mult,
                        op1=ALU.mult,
                    )

            # ---- phase C: OT = sum_k vt_k^T . u_k  (transposed AV) ----
            with tc.tile_pool(name="psC", bufs=1, space="PSUM") as psC:
                ot = psC.tile([HS, T], F32, tag="ot")
                for k in range(NB):
                    for jq in range(T // NJ):
                        nc.tensor.matmul(
                            ot[:, jq * NJ:(jq + 1) * NJ],
                            vt[:, k * HS:(k + 1) * HS],
                            u[:, k * T + jq * NJ: k * T + (jq + 1) * NJ],
                            start=(k == 0),
                            stop=(k == NB - 1),
                        )
                with tc.tile_pool(name="phC", bufs=1) as pC:
                    ot_sb = pC.tile([HS, T], F32, tag="ot_sb")
                    nc.scalar.copy(ot_sb[:], ot[:])
                    nc.sync.dma_start(ot_d[:, :], ot_sb[:])
            ctx_big.__exit__(None, None, None)

    nc.compile()
    _CACHE["nc"] = nc
    return nc


def _prep_inputs(X, dag, Wk, Wq, Wv):
    X = np.asarray(X, dtype=np.float32)
    dag = np.asarray(dag)
    w16 = {
        "wk": np.asarray(Wk, dtype=np.float16),
        "wq": np.asarray(Wq, dtype=np.float16),
        "wv": np.asarray(Wv, dtype=np.float16),
    }
    m16 = (dag != 0).astype(np.float16)
    in_maps = []
    for core in range(8):
        b, h = divmod(core, 2)
        xb = X[b].astype(np.float16)
        in_maps.append(
            {
                "xt": np.ascontiguousarray(xb.T),
                "xtq": np.ascontiguousarray(xb[h * TH:(h + 1) * TH].T),
                "m": np.ascontiguousarray(m16[h * TH:(h + 1) * TH]),
                **w16,
            }
        )
    return in_maps


def kernel(X, dag, Wk, Wq, Wv, _trace=False):
    nc = _build()
    in_maps = _prep_inputs(X, dag, Wk, Wq, Wv)
    res = run_bass_kernel_spmd(nc, in_maps, list(range(8)), trace=_trace)
    out = np.empty((B, T, HS), dtype=np.float32)
    for b in range(B):
        ot = res.results[2 * b]["ot"] + res.results[2 * b + 1]["ot"]
        o = ot.T / np.float32(VSCALE)
        out[b] = o / (1.0 + np.exp(-o))  # swish: o * sigmoid(o)
    if _trace:
        return out, res
    return out

